# revision 40
# baseline (speedup 1.0000x reference)
"""Trainium2 Bass kernel for nn_CELoss_Marginal_Smooth (CE loss with marginal
attention smoothing) on 8 NeuronCores.

Strategy
--------
loss = -mean_i[ (1-w2_i)*x[i,t_i] + w2_i*S_i - (1+11*w2_i)*lse_i ]
  where S_i = sum_c x[i,c], lse_i = log(sum_c exp(x[i,c])), and
  w2_i = (1-ALPHA)*att(t_i) takes one of 3 distinct values (att is 1/#grid-
  neighbors on a 3x4 grid: corners 1/3, edges 1/5, centers 1/8).

The host shards rows across 8 cores AND groups rows by target class inside
each core's shard (the loss is permutation-invariant, so row order is a
sharding/layout choice), ordering the 12 class blocks by weight group
(edges, corners, centers). Each (partition, class) cell is padded with
zero rows to a uniform count qpc. The staged per-core buffer is fp8-e4m3
(2e-2 rel tolerance; measured end-to-end error ~1.5e-4) and
logit-plane-major per block: X[p, b, j, q] = x[row(p,b,q), j], so every
device operand is a contiguous slice:
  - exp          -> ACT, bf16 out. The bottleneck: 1 elem/cycle/partition
                    at 1.2 GHz, dtype-independent, and ACT is the only exp
                    engine. The kernel is ACT-saturated end to end.
  - sumexp       -> DVE pairwise-add tree over contiguous bf16 planes (2x
                    packed mode), hidden under the exp stream
  - S/XT sums    -> PE one-hot-stationary matmuls into per-weight-group
                    PSUM rows, hidden under the exp stream (fp8 rhs)
  - lse          -> 3 deferred ACT ln passes (one per weight group) with
                    the free-dim reduction folded in via accum_out
Head: the first block's load is split into ramped chunks so the first exp
starts as soon as ~90 KB lands. Tail: only the last ln pass depends on the
final block's tree. The device emits 3x3 partials; the host applies exact
fp64 group weights, corrects the pad rows (each contributes exactly ln(12)
to its group's lse sum), and combines the 8 cores.

Probed on HW 2026-08-08: the combined natural_log_exp_and_others ACT table
set returns wrong Ln values; the default two-table selection is kept.
"""
import sys

if "/opt/trn_rl_repo" not in sys.path:
    sys.path.insert(0, "/opt/trn_rl_repo")

import math
from contextlib import ExitStack

import numpy as np

import concourse.bass as bass
import concourse.tile as tile
from concourse import bacc, mybir
from concourse.bass_utils import run_bass_kernel_spmd
from concourse.tile_rust import add_dep_helper

C = 12
P = 128
NCORES = 8
ALPHA = 0.6
MM_CHUNK = 512     # moving free-dim per rect matmul (PSUM bank width)

_F32 = mybir.dt.float32
_BF16 = mybir.dt.bfloat16
_FP8 = mybir.dt.float8e4
_AF = mybir.ActivationFunctionType

# The per-class weights take only 3 distinct values (grid corners, edges,
# centers), so blocks are laid out grouped by weight: blocks 0-5 = edge
# classes, 6-9 = corner classes, 10-11 = center classes. The device then
# only needs 3 weighted partial sums, and the whole lse reduction collapses
# into 3 ln+accumulate ACT instructions.
_PERM = (1, 2, 4, 7, 9, 10, 0, 3, 8, 11, 5, 6)   # block -> class id
_WG_OF_BLOCK = (0,) * 6 + (1,) * 4 + (2,) * 2     # block -> weight group
_WG_BOUNDS = (0, 6, 10, 12)                       # block ranges per group
_NWG = 3

# group layout: (first block, n blocks). The first two groups are single
# blocks so the first exp only waits on a small DMA; the last two are
# single blocks to shorten the tail chain (exp -> tree -> ln -> out).
_GROUPS = [(0, 1), (1, 1), (2, 2), (4, 2), (6, 2), (8, 2), (10, 1), (11, 1)]
# plane-count chunks for the latency-critical head loads: ramp up transfer
# sizes so the first exp starts as early as possible without starving
_HEAD_CHUNKS = {0: (2, 4, 6), 1: (6, 6)}


def _att_values():
    i = np.arange(C)
    r, c = i // 4, i % 4
    up, dn = (r - 1 >= 0), (r + 1 <= 2)
    lf, rt = (c - 1 >= 0), (c + 1 <= 3)
    cnt = (up.astype(np.int32) + dn + lf + rt
           + (up & lf) + (up & rt) + (dn & lf) + (dn & rt))
    return 1.0 / cnt


def _weights():
    att = _att_values()
    w2 = (1.0 - ALPHA) * att          # weight of S_i
    w1 = 1.0 - w2                     # weight of x[i, t_i]
    wl = 1.0 + 11.0 * w2              # weight of lse_i
    return w2, w1, wl


def _group_weights():
    """Per-weight-group weights; constant within each group by symmetry."""
    w2, w1, wl = _weights()
    reps = [_PERM[_WG_BOUNDS[g]] for g in range(_NWG)]
    for b, c in enumerate(_PERM):     # sanity: grouping really is constant
        g = _WG_OF_BLOCK[b]
        assert w2[c] == w2[reps[g]]
    return w2[reps], w1[reps], wl[reps]


def _build(qpc: int):
    """Build + finalize the per-core Bass program for a given qpc."""
    fp = qpc * C                      # free elements per class block
    nc = bacc.Bacc("TRN2", target_bir_lowering=False, debug=False,
                   num_devices=NCORES)
    # NOTE: the combined natural_log_exp_and_others activation table set
    # produces wrong Ln results on hardware (probed 2026-08-08); keep the
    # default per-function table selection (exp_and_others + natural_log).
    x = nc.declare_dram_parameter("x", [P, C * fp], _FP8, isOutput=False)
    out = nc.declare_dram_parameter("out", [_NWG, 3], _F32, isOutput=True)

    with tile.TileContext(nc) as tc, ExitStack() as ctx:
        xp = ctx.enter_context(tc.tile_pool(name="xp", bufs=4))
        ep = ctx.enter_context(tc.tile_pool(name="ep", bufs=2))
        tp = ctx.enter_context(tc.tile_pool(name="tp", bufs=2))
        sp = ctx.enter_context(tc.tile_pool(name="sp", bufs=1))
        pp = ctx.enter_context(tc.tile_pool(name="pp", bufs=1, space="PSUM"))

        # matmul outputs must land at PSUM base partition 0, so weight
        # group g's partial sums are routed to PSUM row g via a one-hot
        # stationary: oneh8[:, g*NWG + g] = 1, rest 0 -> out row g = column
        # sums, rows m != g accumulate zeros
        oneh8 = sp.tile([P, _NWG * _NWG], _FP8)
        nc.vector.memset(oneh8[:], 0.0)
        ohv = oneh8[:].rearrange("p (a b) -> p a b", a=_NWG)
        for g in range(_NWG):
            nc.vector.memset(ohv[:, g, g:g + 1], 1.0)
        ones32 = sp.tile([P, 1], _F32)
        nc.vector.memset(ones32[:], 1.0)
        lacc = sp.tile([P, _NWG], _F32)
        sebuf = sp.tile([P, C * qpc], _BF16)
        lsed = sp.tile([P, C * qpc], _BF16)
        ps_s = pp.tile([_NWG, MM_CHUNK], _F32)
        ps_xt = pp.tile([_NWG, MM_CHUNK], _F32)
        ps_l = pp.tile([_NWG, 1], _F32)

        last_exp = None
        for c0, ng in _GROUPS:
            gf = ng * fp
            xt = xp.tile([P, gf], _FP8, tag="x")
            et = ep.tile([P, gf], _BF16, tag="e")
            if c0 in _HEAD_CHUNKS:
                # split the head loads so the exp pipeline starts as soon as
                # a small chunk lands (DMA fixed latency dominates)
                off = 0
                for planes in _HEAD_CHUNKS[c0]:
                    h = planes * qpc
                    nc.sync.dma_start(xt[:, off:off + h],
                                      x[:, c0 * fp + off:c0 * fp + off + h])
                    last_exp = nc.scalar.activation(et[:, off:off + h],
                                                    xt[:, off:off + h],
                                                    _AF.Exp)
                    off += h
                assert off == gf
            else:
                nc.sync.dma_start(xt[:], x[:, c0 * fp:c0 * fp + gf])
                last_exp = nc.scalar.activation(et[:], xt[:], _AF.Exp)

            # pairwise-add tree over the 12 logit planes of each class in
            # the group; all operands are contiguous bf16 runs of qpc (2x
            # packed DVE mode)
            ev = et[:].rearrange("p (t j q) -> p t j q", t=ng, j=C)
            t6 = tp.tile([P, ng, 6, qpc], _BF16, tag="t6")
            nc.vector.tensor_add(t6[:], ev[:, :, 0:6], ev[:, :, 6:12])
            t3 = tp.tile([P, ng, 3, qpc], _BF16, tag="t3")
            nc.vector.tensor_add(t3[:], t6[:, :, 0:3], t6[:, :, 3:6])
            t1 = tp.tile([P, ng, 1, qpc], _BF16, tag="t1")
            nc.vector.tensor_add(t1[:], t3[:, :, 0:1], t3[:, :, 1:2])
            sev = sebuf[:, c0 * qpc:(c0 + ng) * qpc].rearrange(
                "p (t j q) -> p t j q", t=ng, j=1)
            nc.vector.tensor_add(sev, t1[:], t3[:, :, 2:3])

            for u in range(ng):
                b = c0 + u
                wg = _WG_OF_BLOCK[b]
                lh = oneh8[:, wg * _NWG:(wg + 1) * _NWG]
                # S: sum of the whole block, accumulated column-wise into
                # the block's weight-group row
                xflat = xt[:, u * fp:(u + 1) * fp]
                for i in range(0, fp, MM_CHUNK):
                    w = min(MM_CHUNK, fp - i)
                    nc.tensor.matmul(ps_s[:, 0:w], lhsT=lh,
                                     rhs=xflat[:, i:i + w],
                                     start=(b == 0 and i == 0),
                                     stop=(b == C - 1 and i + MM_CHUNK >= fp))
                # XT: sum of the block's own-logit plane
                pl = _PERM[b]
                xplane = xt[:, u * fp + pl * qpc:u * fp + (pl + 1) * qpc]
                nc.tensor.matmul(ps_xt[:, 0:qpc], lhsT=lh, rhs=xplane,
                                 start=(b == 0), stop=(b == C - 1))

        fin = sp.tile([_NWG, 3], _F32)
        # S/XT accumulation groups close with block 11's matmuls, which only
        # depend on the last DMA -- these reduces overlap the ln stream
        nc.vector.tensor_reduce(fin[:, 0:1], ps_s[:, :],
                                axis=mybir.AxisListType.X,
                                op=mybir.AluOpType.add)
        nc.vector.tensor_reduce(fin[:, 1:2], ps_xt[:, 0:qpc],
                                axis=mybir.AxisListType.X,
                                op=mybir.AluOpType.add)

        # deferred: lse = ln(sumexp), one ACT pass per weight group with the
        # free-dim reduction folded into the instruction via accum_out; the
        # center group (loaded last) comes last so only it waits on the
        # final tree
        for g in range(_NWG):
            lo, hi = _WG_BOUNDS[g] * qpc, _WG_BOUNDS[g + 1] * qpc
            ln_inst = nc.scalar.activation(lsed[:, lo:hi], sebuf[:, lo:hi],
                                           _AF.Ln,
                                           accum_out=lacc[:, g:g + 1])
            add_dep_helper(ln_inst.ins, last_exp.ins, False,
                           "ln after all exps (act stream ordering)")
        # partition-sum of lacc -> per-weight-group lse totals
        nc.tensor.matmul(ps_l[:, 0:1], lhsT=lacc[:], rhs=ones32[:],
                         start=True, stop=True)
        nc.vector.tensor_copy(fin[:, 2:3], ps_l[:, 0:1])
        nc.sync.dma_start(out[:], fin[:])
    nc.finalize()
    return nc


_PROG_CACHE: dict = {}
_LAST_IN_MAPS = None


def _program(qpc: int):
    if qpc not in _PROG_CACHE:
        _PROG_CACHE[qpc] = _build(qpc)
    return _PROG_CACHE[qpc]


def kernel(outputs: np.ndarray, targets: np.ndarray) -> np.ndarray:
    x = np.asarray(outputs)
    t = np.asarray(targets).astype(np.int64, copy=False).ravel()
    B = x.shape[0]
    assert x.shape == (B, C)

    counts = np.bincount(t, minlength=C)
    slots = NCORES * P
    # uniform per-(partition, class) row count; even count keeps every
    # staged bf16 plane 4-byte aligned in the free dim
    qpc = max(64, 2 * math.ceil(counts.max() / (slots * 2)))

    # block-major index layout (block b holds class _PERM[b]):
    # A[b, s, q] = global row (or -1 pad)
    A = np.full((C, slots * qpc), -1, dtype=np.int64)
    order = np.argsort(t, kind="stable")
    bounds = np.concatenate(([0], np.cumsum(counts)))
    for b in range(C):
        c = _PERM[b]
        A[b, :counts[c]] = order[bounds[c]:bounds[c + 1]]
    A = A.reshape(C, slots, qpc)

    xb = x.astype(mybir.dt.np(_FP8))
    in_maps = []
    for k in range(NCORES):
        idx = A[:, k * P:(k + 1) * P, :]          # [C, P, qpc]
        g = xb[idx.clip(min=0)]                   # [C, P, qpc, 12]
        g[idx < 0] = 0
        # plane-major within each block: [p, b, j, q]
        xk = np.ascontiguousarray(g.transpose(1, 0, 3, 2)).reshape(P, -1)
        in_maps.append({"x": xk})

    nc = _program(qpc)
    global _LAST_IN_MAPS
    _LAST_IN_MAPS = in_maps

    # guard against transient execution flakes: rerun until two consecutive
    # runs agree on the combined partials
    prev = None
    for _ in range(4):
        res = run_bass_kernel_spmd(nc, in_maps, list(range(NCORES)))
        acc = np.zeros((_NWG, 3), dtype=np.float64)
        for k in range(NCORES):
            acc += np.asarray(res.results[k]["out"]).astype(np.float64)
        if prev is not None and np.allclose(acc, prev, rtol=1e-3, atol=10.0):
            break
        prev = acc
    s_g, xt_g, l_g = acc[:, 0], acc[:, 1], acc[:, 2]

    w2g, w1g, wlg = _group_weights()
    npad_c = qpc * slots - counts
    npad_g = np.array([
        sum(npad_c[_PERM[b]] for b in range(_WG_BOUNDS[g], _WG_BOUNDS[g + 1]))
        for g in range(_NWG)])
    l_g = l_g - npad_g * math.log(12.0)           # pad rows: se = 12 exactly
    partial = (w1g * xt_g + w2g * s_g - wlg * l_g).sum()
    loss = -partial / B
    return np.float32(loss)


if __name__ == "__main__":
    rng = np.random.default_rng(1)
    Bs = 4194304
    xs = rng.standard_normal((Bs, C)).astype(np.float32)
    ts = rng.integers(0, C, size=Bs).astype(np.int64)
    print("loss:", kernel(xs, ts))
